# revision 1
# baseline (speedup 1.0000x reference)
"""AFeFET quantized linear layer on 8 TRN2 NeuronCores.

Reference computation:
  qv   = snap(4.5*(1 + w*a)) to nearest of {3.5,4.0,4.5,5.0,5.5}
  qw   = (qv/4.5 - 1)/a * exp(-1e-3) * (1 - clip(wc/1e8*0.1, 0, 0.5))
  y    = x @ qw.T          x:[8,2048,4096] f32, w:[4096,4096] f32, wc int64

Sharding: batch 2-way x out_features 4-way (8 cores).  Each core:
  xlin [8192,4096] f32r (tile-linearized transpose of its 4 batches)
  wT   [4096,1024] f32, wcT [4096,1024] int32, alpha [1,1] f32
  y    [8192,1024] f32

Device does the whole quantization chain (DVE+ACT) and the GEMM in
float32r (fp32 storage, ~1e-4 matmul precision, 4x the fp32 rate).
Host does only dtype-preserving layout prep (transpose/reshape) and the
lossless int64->int32 narrowing (values < 2^31).
"""
import sys
sys.path.insert(0, "/opt/trn_rl_repo")
import numpy as np

import concourse.bass as bass
import concourse.mybir as mybir
import concourse.tile as tile
from concourse import bacc
from concourse.bass_utils import run_bass_kernel_spmd

P = 128
N_CORES = 8

# full-problem shape
B, S, IN_F, OUT_F = 8, 2048, 4096, 4096
BATCH_WAYS, OUT_WAYS = 2, 4
TOK = (B // BATCH_WAYS) * S          # 8192 tokens per core
O = OUT_F // OUT_WAYS                # 1024 out_features per core

C_DECAY = float(np.exp(np.float64(-0.001)) / 4.5)   # 0.22200011107408333


def build(tok=TOK, kin=IN_F, o=O, g_width=512, early_split=0, quant_halves=1, nxd=4, wdma=2, xbufs=2, wbufs=2, ybufs=4):
    """Per-core SPMD graph. tok/kin multiples of 128, o multiple of g_width."""
    ksub = kin // P
    ntok = tok // P
    ngrp = o // g_width

    nc = bacc.Bacc("TRN2", target_bir_lowering=False, debug=False)
    xlin = nc.dram_tensor("xlin", [tok, kin], mybir.dt.float32r, kind="ExternalInput")
    wt = nc.dram_tensor("wt", [kin, o], mybir.dt.float32, kind="ExternalInput")
    wct = nc.dram_tensor("wct", [kin, o], mybir.dt.int32, kind="ExternalInput")
    alpha = nc.dram_tensor("alpha", [1, 1], mybir.dt.float32, kind="ExternalInput")
    y = nc.dram_tensor("y", [tok, o], mybir.dt.float32, kind="ExternalOutput")

    # xlin row t*P+p, col ks*P+c holds x.T[ks*P+p, t*P+c]: tile t DMAs as one
    # contiguous [P, kin] block straight into SBUF layout.
    xr = xlin.ap().rearrange("(t p) c -> t p c", p=P)

    with tile.TileContext(nc) as tc:
        with (
            tc.tile_pool(name="const", bufs=1) as constp,
            tc.tile_pool(name="wstage", bufs=wbufs) as wstage,
            tc.tile_pool(name="qpool", bufs=1) as qpool,
            tc.tile_pool(name="tmp", bufs=1) as tmpp,
            tc.tile_pool(name="xpool", bufs=xbufs) as xpool,
            tc.tile_pool(name="ypool", bufs=ybufs) as ypool,
            tc.tile_pool(name="ps", bufs=4, space="PSUM") as ps,
        ):
            # ---- alpha-derived per-partition scalars (one padded tile) ----
            cs = constp.tile([P, 8], mybir.dt.float32)
            a_sb, s9a, rec, crec, c2 = (cs[:, i:i + 1] for i in range(5))
            nc.vector.memset(c2, 2.0)
            alpha_bcast = bass.AP(tensor=alpha, offset=0, ap=[[0, P], [1, 1]])
            nc.gpsimd.dma_start(out=a_sb, in_=alpha_bcast)
            nc.vector.tensor_scalar_mul(s9a, a_sb, 9.0)
            nc.vector.reciprocal(rec, a_sb)
            nc.vector.tensor_scalar_mul(crec, rec, 0.5 * C_DECAY)  # C/(2a) folded into y

            # ---- prefetch x for the first tok-tiles so PE can start the
            # moment the first qw strips land ----
            NXD = nxd
            xq = max(P, (ksub * P) // NXD)
            nxpre = min(2, tok // P)
            xpre = []
            for t in range(nxpre):
                xt = xpool.tile([P, ksub * P], mybir.dt.float32r,
                                name=f"xpre{t}", tag="xt")
                for h in range((ksub * P) // xq):
                    nc.sync.dma_start(
                        xt[:, h * xq:(h + 1) * xq],
                        xr[t][:, h * xq:(h + 1) * xq])
                xpre.append(xt)

            # ---- quantization chain -> resident qwT [P, ksub, o] f32r ----
            qw = qpool.tile([P, ksub, o], mybir.dt.float32r)
            wtr = wt.ap().rearrange("(ks p) o -> ks p o", p=P)
            wctr = wct.ap().rearrange("(ks p) o -> ks p o", p=P)
            # column-half-outer order: all ks of out-cols [0:o/2) first, so
            # the early g-split tok-tiles can start after half the quant.
            oh = o // quant_halves
            for hq in range(quant_halves):
                osl = slice(hq * oh, (hq + 1) * oh)
                for ks in range(ksub):
                    # several dma_starts per strip -> more DMA queues in flight
                    ohh = oh // wdma if oh >= 128 * wdma else oh
                    w_s = wstage.tile([P, oh], mybir.dt.float32, name="w_s", tag="w_s")
                    wc_s = wstage.tile([P, oh], mybir.dt.int32, name="wc_s", tag="wc_s")
                    for j in range(oh // ohh):
                        jsl = slice(j * ohh, (j + 1) * ohh)
                        gsl = slice(hq * oh + j * ohh, hq * oh + (j + 1) * ohh)
                        nc.sync.dma_start(w_s[:, jsl], wtr[ks][:, gsl])
                        nc.sync.dma_start(wc_s[:, jsl], wctr[ks][:, gsl])
                    # t1 = w*9a + 2 (ACT, AP scale/bias path)
                    t1 = tmpp.tile([P, oh], mybir.dt.float32, name="t1", tag="t1")
                    nc.scalar.activation(t1[:], w_s[:],
                                         mybir.ActivationFunctionType.Identity,
                                         bias=c2, scale=s9a[:])
                    # u = rint(clip(t1, 0, 4))  (clip in f32, RNE on int32 write)
                    u = tmpp.tile([P, oh], mybir.dt.int32, name="u", tag="u")
                    nc.vector.tensor_scalar(u[:], t1[:], 4.0, 0.0,
                                            op0=mybir.AluOpType.min,
                                            op1=mybir.AluOpType.max)
                    # m = 1 - 1e-9*wc (ACT, immediate scale)
                    m_f = tmpp.tile([P, oh], mybir.dt.float32, name="m_f", tag="m_f")
                    nc.scalar.activation(m_f[:], wc_s[:],
                                         mybir.ActivationFunctionType.Identity,
                                         bias=1.0, scale=-1e-9)
                    # qw' = (u - 2) * m ; the x0.5*C/a lives in the y-copy scale
                    nc.vector.scalar_tensor_tensor(qw[:, ks, osl], u[:], -2.0, m_f[:],
                                                   op0=mybir.AluOpType.add,
                                                   op1=mybir.AluOpType.mult)

            # ---- GEMM: y[t*P:(t+1)*P, g*512:(g+1)*512] ----
            for t in range(ntok):
                if t < nxpre:
                    xt = xpre[t]
                else:
                    xt = xpool.tile([P, ksub * P], mybir.dt.float32r,
                                    name="xt", tag="xt")
                    for h in range((ksub * P) // xq):
                        nc.sync.dma_start(
                            xt[:, h * xq:(h + 1) * xq],
                            xr[t][:, h * xq:(h + 1) * xq])
                pts = []
                for g in range(ngrp):
                    pt = ps.tile([P, g_width], mybir.dt.float32,
                                 name=f"acc{g}", tag=f"acc{g}")
                    pts.append(pt)
                if t < early_split and ngrp > 1:
                    for g in range(ngrp):
                        for ks in range(ksub):
                            nc.tensor.matmul(pts[g][:], xt[:, ks * P:(ks + 1) * P],
                                             qw[:, ks, g * g_width:(g + 1) * g_width],
                                             start=(ks == 0), stop=(ks == ksub - 1))
                else:
                    for ks in range(ksub):
                        lhsT = xt[:, ks * P:(ks + 1) * P]
                        for g in range(ngrp):
                            nc.tensor.matmul(pts[g][:], lhsT,
                                             qw[:, ks, g * g_width:(g + 1) * g_width],
                                             start=(ks == 0), stop=(ks == ksub - 1))
                for g in range(ngrp):
                    yt = ypool.tile([P, g_width], mybir.dt.float32, name="yt", tag="yt")
                    nc.scalar.activation(yt[:], pts[g][:],
                                         mybir.ActivationFunctionType.Identity,
                                         bias=0.0, scale=crec[:])
                    nc.sync.dma_start(
                        y.ap()[t * P:(t + 1) * P, g * g_width:(g + 1) * g_width],
                        yt[:])
    nc.finalize()
    return nc


def _prep_x(xs):
    """[tok, kin] f32 -> tile-linearized [tok, kin] where row t*P+p holds
    x.T[128ks+p, 128t+col] at col ks*P+col (SBUF DMA order)."""
    tok, kin = xs.shape
    nt, ks = tok // P, kin // P
    # want out[t, p, ks, col] = xs[t*P+col, ks*P+p]
    return np.ascontiguousarray(
        xs.reshape(nt, P, ks, P).transpose(0, 3, 2, 1).reshape(tok, kin))


_NC_CACHE = {}


def prep_in_maps(x, weight, alpha, write_count):
    x = np.asarray(x)
    weight = np.asarray(weight)
    alpha = np.asarray(alpha)
    write_count = np.asarray(write_count)
    a11 = alpha.reshape(1, 1).astype(np.float32)
    in_maps = []
    xl = {}
    for b in range(BATCH_WAYS):
        xs = x[b * (B // BATCH_WAYS):(b + 1) * (B // BATCH_WAYS)].reshape(TOK, IN_F)
        xl[b] = _prep_x(np.ascontiguousarray(xs))
    for c in range(N_CORES):
        b, q = divmod(c, OUT_WAYS)
        wT = np.ascontiguousarray(weight[q * O:(q + 1) * O, :].T)       # [IN_F, O]
        wcT = np.ascontiguousarray(
            write_count[q * O:(q + 1) * O, :].T).astype(np.int32)
        in_maps.append({"xlin": xl[b], "wt": wT, "wct": wcT, "alpha": a11})
    return in_maps


def assemble(results):
    """results: list of 8 per-core dicts with 'y' [TOK, O]."""
    y = np.empty((B * S, OUT_F), dtype=np.float32)
    for c in range(N_CORES):
        b, q = divmod(c, OUT_WAYS)
        y[b * TOK:(b + 1) * TOK, q * O:(q + 1) * O] = results[c]["y"]
    return y.reshape(B, S, OUT_F)


def kernel(x, weight, alpha, write_count):
    if "full" not in _NC_CACHE:
        _NC_CACHE["full"] = build()
    nc = _NC_CACHE["full"]
    in_maps = prep_in_maps(x, weight, alpha, write_count)
    last_err = None
    for attempt in range(3):
        try:
            res = run_bass_kernel_spmd(nc, in_maps, core_ids=list(range(N_CORES)))
            return assemble(res.results)
        except Exception as e:  # transient NRT_EXEC_UNIT_UNRECOVERABLE etc.
            last_err = e
            import time as _time
            _time.sleep(10)
    raise last_err



# revision 2
# speedup vs baseline: 1.2031x; 1.2031x over previous
"""AFeFET quantized linear layer on 8 TRN2 NeuronCores — v2 (bf16 GEMM).

Reference computation:
  qv   = snap(4.5*(1 + w*a)) to nearest of {3.5,4.0,4.5,5.0,5.5}
  qw   = (qv/4.5 - 1)/a * exp(-1e-3) * (1 - clip(wc/1e8*0.1, 0, 0.5))
  y    = x @ qw.T          x:[8,2048,4096] f32, w:[4096,4096] f32, wc int64

Sharding: batch 2-way x out_features 4-way (8 cores).  Each core:
  xlin [8192,4096] f32 (tile-linearized transpose of its 4 batches)
  wT   [4096,1024] f32, wcT [4096,1024] int32, alpha [1,1] f32
  y    [8192,1024] f32

v2 vs baseline: measured HW PE clock is ~1.88 GHz under 8-core load, so
a N=512 matmul costs ~273 ns (bf16) / ~281 ns (f32r) regardless of
weight reloads (LDWEIGHTS is overlapped).  GEMM floor ~1120 us/core in
bf16.  The baseline lost ~110 us to the serial quant preamble; here:
  - x is staged f32 (sync-queue DMA) and cast to bf16 on DVE; qw is
    produced in bf16 by the quant chain (error budget ~2.4e-3 rel).
  - quant runs in column halves (all 32 k-strips of out-cols [0:512)
    first); the first 8 token tiles run as two 8-chain PSUM generations
    interleaved ks-outer, consuming each fresh strip 8x, so the PE
    stays busy through the whole 94 us w/wc DMA.
  - all heavy DMAs share the sync queue and are emitted in explicit
    order to apportion HBM bandwidth; quant math runs on DVE+GpSimd so
    ACT only drains PSUM.
"""
import sys
sys.path.insert(0, "/opt/trn_rl_repo")
import numpy as np

import concourse.bass as bass
import concourse.mybir as mybir
import concourse.tile as tile
from concourse import bacc
from concourse.bass_utils import run_bass_kernel_spmd

P = 128
N_CORES = 8

# full-problem shape
B, S, IN_F, OUT_F = 8, 2048, 4096, 4096
BATCH_WAYS, OUT_WAYS = 2, 4
TOK = (B // BATCH_WAYS) * S          # 8192 tokens per core
O = OUT_F // OUT_WAYS                # 1024 out_features per core

C_DECAY = float(np.exp(np.float64(-0.001)) / 4.5)   # 0.22200011107408333


def build(tok=TOK, kin=IN_F, o=O, ngen=8, xbufs=11, xsbufs=3, ybufs=4):
    """Per-core SPMD graph. tok/kin multiples of 128, o multiple of 1024."""
    ksub = kin // P          # 32 k-strips
    ntok = tok // P          # 64 token tiles
    oh = o // 2              # column half width (512)
    xh = (ksub * P) // 2     # x tile half width (2048)

    nc = bacc.Bacc("TRN2", target_bir_lowering=False, debug=False)
    xlin = nc.dram_tensor("xlin", [tok, kin], mybir.dt.float32, kind="ExternalInput")
    wt = nc.dram_tensor("wt", [kin, o], mybir.dt.float32, kind="ExternalInput")
    wct = nc.dram_tensor("wct", [kin, o], mybir.dt.int32, kind="ExternalInput")
    alpha = nc.dram_tensor("alpha", [1, 1], mybir.dt.float32, kind="ExternalInput")
    y = nc.dram_tensor("y", [tok, o], mybir.dt.float32, kind="ExternalOutput")

    # xlin row t*P+p, col ks*P+c holds x.T[ks*P+p, t*P+c]: tile t DMAs as one
    # contiguous [P, kin] block straight into SBUF layout.
    xr = xlin.ap().rearrange("(t p) c -> t p c", p=P)
    wtr = wt.ap().rearrange("(ks p) o -> ks p o", p=P)
    wctr = wct.ap().rearrange("(ks p) o -> ks p o", p=P)

    with tile.TileContext(nc) as tc:
        with (
            tc.tile_pool(name="const", bufs=1) as constp,
            tc.tile_pool(name="wstage", bufs=2) as wstage,
            tc.tile_pool(name="qpool", bufs=1) as qpool,
            tc.tile_pool(name="tmp", bufs=2) as tmpp,
            tc.tile_pool(name="xstage", bufs=xsbufs) as xstage,
            tc.tile_pool(name="xpool", bufs=xbufs) as xpool,
            tc.tile_pool(name="ypool", bufs=ybufs) as ypool,
            tc.tile_pool(name="ps", bufs=8, space="PSUM") as ps,
        ):
            # ---- alpha-derived per-partition scalars (one padded tile) ----
            cs = constp.tile([P, 8], mybir.dt.float32)
            a_sb, s9a, rec, crec = (cs[:, i:i + 1] for i in range(4))
            alpha_bcast = bass.AP(tensor=alpha, offset=0, ap=[[0, P], [1, 1]])
            nc.gpsimd.dma_start(out=a_sb, in_=alpha_bcast)
            nc.vector.tensor_scalar_mul(s9a, a_sb, 9.0)
            nc.vector.reciprocal(rec, a_sb)
            nc.vector.tensor_scalar_mul(crec, rec, 0.5 * C_DECAY)  # C/(2a) in y-copy

            qw = qpool.tile([P, ksub, o], mybir.dt.bfloat16)

            xt_tiles = {}

            def emit_x(t):
                """Stage f32 halves on the sync queue, cast to bf16 on DVE."""
                xb = xpool.tile([P, ksub * P], mybir.dt.bfloat16,
                                name=f"x{t}", tag="xt")
                for h in range(2):
                    hs = slice(h * xh, (h + 1) * xh)
                    xs = xstage.tile([P, xh], mybir.dt.float32, name="xs", tag="xs")
                    nc.sync.dma_start(xs[:], xr[t][:, hs])
                    nc.vector.tensor_scalar_mul(xb[:, hs], xs[:], 1.0)
                xt_tiles[t] = xb

            def emit_quant(mh, ks):
                osl = slice(mh * oh, (mh + 1) * oh)
                w_s = wstage.tile([P, oh], mybir.dt.float32, name="w_s", tag="w_s")
                wc_s = wstage.tile([P, oh], mybir.dt.int32, name="wc_s", tag="wc_s")
                nc.sync.dma_start(w_s[:], wtr[ks][:, osl])
                nc.sync.dma_start(wc_s[:], wctr[ks][:, osl])
                # t1 = w*9a + 2  (DVE, per-partition scalar AP)
                t1 = tmpp.tile([P, oh], mybir.dt.float32, name="t1", tag="t1")
                nc.vector.tensor_scalar(t1[:], w_s[:], s9a, 2.0,
                                        op0=mybir.AluOpType.mult,
                                        op1=mybir.AluOpType.add)
                # u = rint(clip(t1, 0, 4))  (clip in f32, RNE on int32 write)
                u = tmpp.tile([P, oh], mybir.dt.int32, name="u", tag="u")
                nc.vector.tensor_scalar(u[:], t1[:], 4.0, 0.0,
                                        op0=mybir.AluOpType.min,
                                        op1=mybir.AluOpType.max)
                # m = 1 - 1e-9*wc  (GpSimd so ACT stays drain-only)
                m_f = tmpp.tile([P, oh], mybir.dt.float32, name="m_f", tag="m_f")
                nc.gpsimd.tensor_scalar(m_f[:], wc_s[:], -1e-9, 1.0,
                                        op0=mybir.AluOpType.mult,
                                        op1=mybir.AluOpType.add)
                # qw' = (u - 2) * m in bf16; the 0.5*C/a lives in the y-copy
                nc.vector.scalar_tensor_tensor(qw[:, ks, osl], u[:], -2.0, m_f[:],
                                               op0=mybir.AluOpType.add,
                                               op1=mybir.AluOpType.mult)

            def emit_gen_mms(ts, gs):
                """Interleaved ks-outer accumulation chains for tiles `ts` x
                column groups `gs`; returns psum tiles keyed (t, g)."""
                pts = {}
                for t in ts:
                    for g in gs:
                        pts[(t, g)] = ps.tile([P, oh], mybir.dt.float32,
                                              name=f"acc{t}_{g}", tag="acc")
                for ks in range(ksub):
                    for t in ts:
                        lhsT = xt_tiles[t][:, ks * P:(ks + 1) * P]
                        for g in gs:
                            nc.tensor.matmul(pts[(t, g)][:], lhsT,
                                             qw[:, ks, g * oh:(g + 1) * oh],
                                             start=(ks == 0), stop=(ks == ksub - 1))
                return pts

            def emit_drains(pts):
                for (t, g), pt in pts.items():
                    yt = ypool.tile([P, oh], mybir.dt.float32, name="yt", tag="yt")
                    nc.scalar.activation(yt[:], pt[:],
                                         mybir.ActivationFunctionType.Identity,
                                         bias=0.0, scale=crec[:])
                    nc.sync.dma_start(
                        y.ap()[t * P:(t + 1) * P, g * oh:(g + 1) * oh], yt[:])

            # ---- PE warmup: dummy matmuls while the first DMAs land, so
            # the HAM clock gate is fully ramped when real work arrives ----
            wm = constp.tile([P, 3 * P], mybir.dt.bfloat16)
            nc.vector.memset(wm[:], 0.0)
            pw = ps.tile([P, oh], mybir.dt.float32, name="warm", tag="acc")
            for i in range(24):
                nc.tensor.matmul(pw[:, 0:2 * P], wm[:, 0:P], wm[:, P:3 * P],
                                 start=(i == 0), stop=(i == 23))

            # ---- phase A: first two x tiles ----
            emit_x(0)
            emit_x(1)
            # ---- phase B: quant mh0, x2..x7 spread between strips ----
            xa = 2
            for ks in range(ksub):
                emit_quant(0, ks)
                if ks % 4 == 3 and xa < ngen:
                    emit_x(xa)
                    xa += 1
            # ---- gen0: tiles 0..7, column group 0 ----
            pts0 = emit_gen_mms(range(ngen), [0])
            # ---- phase C: quant mh1, x8..x11 spread between strips ----
            for ks in range(ksub):
                emit_quant(1, ks)
                if ks % 8 == 7 and xa < ngen + 4:
                    emit_x(xa)
                    xa += 1
            emit_drains(pts0)
            # ---- gen1: tiles 0..7, column group 1 ----
            pts1 = emit_gen_mms(range(ngen), [1])
            emit_drains(pts1)
            # ---- gen2: tiles 8..11, both groups ----
            pts2 = emit_gen_mms(range(ngen, ngen + 4), [0, 1])
            emit_drains(pts2)
            # ---- steady state: x emitted 2 tiles ahead so the sync engine
            # never parks x prefetch behind a drain-gated y DMA ----
            emit_x(ngen + 4)
            emit_x(ngen + 5)
            for t in range(ngen + 4, ntok):
                if t + 2 < ntok:
                    emit_x(t + 2)
                xt = xt_tiles[t]
                pts = [ps.tile([P, oh], mybir.dt.float32,
                               name=f"acc{t}_{g}", tag="acc") for g in (0, 1)]
                for ks in range(ksub):
                    lhsT = xt[:, ks * P:(ks + 1) * P]
                    for g in (0, 1):
                        nc.tensor.matmul(pts[g][:], lhsT,
                                         qw[:, ks, g * oh:(g + 1) * oh],
                                         start=(ks == 0), stop=(ks == ksub - 1))
                emit_drains({(t, g): pts[g] for g in (0, 1)})
    nc.finalize()
    return nc


def _prep_x(xs):
    """[tok, kin] f32 -> tile-linearized [tok, kin] where row t*P+p holds
    x.T[128ks+p, 128t+col] at col ks*P+col (SBUF DMA order)."""
    tok, kin = xs.shape
    nt, ks = tok // P, kin // P
    # want out[t, p, ks, col] = xs[t*P+col, ks*P+p]
    return np.ascontiguousarray(
        xs.reshape(nt, P, ks, P).transpose(0, 3, 2, 1).reshape(tok, kin))


_NC_CACHE = {}


def prep_in_maps(x, weight, alpha, write_count):
    x = np.asarray(x)
    weight = np.asarray(weight)
    alpha = np.asarray(alpha)
    write_count = np.asarray(write_count)
    a11 = alpha.reshape(1, 1).astype(np.float32)
    in_maps = []
    xl = {}
    for b in range(BATCH_WAYS):
        xs = x[b * (B // BATCH_WAYS):(b + 1) * (B // BATCH_WAYS)].reshape(TOK, IN_F)
        xl[b] = _prep_x(np.ascontiguousarray(xs))
    for c in range(N_CORES):
        b, q = divmod(c, OUT_WAYS)
        wT = np.ascontiguousarray(weight[q * O:(q + 1) * O, :].T)       # [IN_F, O]
        wcT = np.ascontiguousarray(
            write_count[q * O:(q + 1) * O, :].T).astype(np.int32)
        in_maps.append({"xlin": xl[b], "wt": wT, "wct": wcT, "alpha": a11})
    return in_maps


def assemble(results):
    """results: list of 8 per-core dicts with 'y' [TOK, O]."""
    y = np.empty((B * S, OUT_F), dtype=np.float32)
    for c in range(N_CORES):
        b, q = divmod(c, OUT_WAYS)
        y[b * TOK:(b + 1) * TOK, q * O:(q + 1) * O] = results[c]["y"]
    return y.reshape(B, S, OUT_F)


def kernel(x, weight, alpha, write_count):
    if "full" not in _NC_CACHE:
        _NC_CACHE["full"] = build()
    nc = _NC_CACHE["full"]
    in_maps = prep_in_maps(x, weight, alpha, write_count)
    last_err = None
    for attempt in range(3):
        try:
            res = run_bass_kernel_spmd(nc, in_maps, core_ids=list(range(N_CORES)))
            return assemble(res.results)
        except Exception as e:  # transient NRT_EXEC_UNIT_UNRECOVERABLE etc.
            last_err = e
            import time as _time
            _time.sleep(10)
    raise last_err
